# revision 17
# baseline (speedup 1.0000x reference)
import numpy as np

import concourse.bacc as bacc
import concourse.bass as bass
import concourse.tile as tile
from concourse import mybir
from concourse.bass_utils import run_bass_kernel_spmd

F32 = mybir.dt.float32
F32R = mybir.dt.float32r
AF = mybir.ActivationFunctionType
ALU = mybir.AluOpType

H, KD, VD = 8, 64, 64
D, DF = 512, 2048
S1, S2, S3 = 768, 1024, 768
P = 128
NCH = D // P
W = 1.25
INV_SQRT = 0.125
EPS = 1e-5

_PROGRAM_CACHE = {}


def _regions(n):
    out = []
    s = 0
    while s < n:
        e = min(s + 512, n)
        out.append((s, e))
        s = e
    return out


class _Ctx:
    pass


def _emit_proj(nc, psum_pool, wT_sb, xT_sb, n_out, n_seq, k_chunks, consume):
    for m in range(n_out // P):
        ps = psum_pool.tile([P, 1024], F32, tag="proj")
        for (a, b) in _regions(n_seq):
            for ko in range(k_chunks):
                nc.tensor.matmul(
                    ps[:, a:b],
                    wT_sb[:, ko, m * P:(m + 1) * P],
                    xT_sb[:, ko, a:b],
                    start=(ko == 0), stop=(ko == k_chunks - 1),
                )
        consume(m, ps[:, :n_seq])


def _emit_ln(nc, cx, psum_pool, z_src, resid, bias_pm, y_sb, n_seq, scale_gb):
    sb, dram = cx.sb, cx.dram
    z_sb = sb.tile([P, NCH, n_seq], F32R, tag="ln_z")
    stat_z = psum_pool.tile([1, 768], F32, tag="ln_stat_z", bufs=1)
    stat_zsq = psum_pool.tile([1, 768], F32, tag="ln_stat_zsq", bufs=1)
    for m in range(NCH):
        ps = z_src(m)
        if bias_pm is not None:
            nc.vector.scalar_tensor_tensor(
                z_sb[:, m, :], ps, bias_pm[:, m:m + 1], resid[:, m, :],
                op0=ALU.add, op1=ALU.add)
        else:
            nc.vector.tensor_tensor(z_sb[:, m, :], ps, resid[:, m, :], ALU.add)
        zsq = cx.zsq_pool.tile([P, 768], F32R, tag="ln_zsq")
        nc.gpsimd.tensor_tensor(zsq[:, :n_seq], z_sb[:, m, :], z_sb[:, m, :],
                                ALU.mult)
        for (a, b) in _regions(n_seq):
            nc.tensor.matmul(stat_z[0:1, a:b], cx.ones_sb[:, 0:1],
                             z_sb[:, m, a:b],
                             start=(m == 0), stop=(m == NCH - 1))
            nc.tensor.matmul(stat_zsq[0:1, a:b], cx.ones_sb[:, 0:1],
                             zsq[:, a:b],
                             start=(m == 0), stop=(m == NCH - 1))
    mean_t = sb.tile([1, n_seq], F32, tag="ln_mean")
    ms_t = sb.tile([1, n_seq], F32, tag="ln_ms")
    nc.vector.tensor_scalar_mul(mean_t[:], stat_z[0:1, :n_seq], 1.0 / D)
    nc.vector.tensor_scalar_mul(ms_t[:], stat_zsq[0:1, :n_seq], 1.0 / D)
    msq = sb.tile([1, n_seq], F32, tag="ln_msq")
    nc.vector.tensor_tensor(msq[:], mean_t[:], mean_t[:], ALU.mult)
    rstd_t = sb.tile([1, n_seq], F32, tag="ln_rstd")
    mr_t = sb.tile([1, n_seq], F32, tag="ln_mr")
    nc.vector.tensor_tensor(rstd_t[:], ms_t[:], msq[:], ALU.subtract)
    nc.scalar.activation(rstd_t[:], rstd_t[:], AF.Ln, bias=cx.eps_sb[0:1, :],
                         scale=1.0)
    nc.scalar.activation(rstd_t[:], rstd_t[:], AF.Exp, bias=0.0, scale=-0.5)
    nc.vector.tensor_tensor(mr_t[:], mean_t[:], rstd_t[:], ALU.mult)
    dln = dram.tile([2, n_seq], F32, tag="ln_dram")
    nc.sync.dma_start(dln[0:1, :], rstd_t[:])
    nc.sync.dma_start(dln[1:2, :], mr_t[:])
    rstd_bc = sb.tile([P, n_seq], F32, tag="ln_rstd_bc")
    mr_bc = sb.tile([P, n_seq], F32, tag="ln_mr_bc")
    nc.gpsimd.dma_start(rstd_bc[:], dln[0:1, :].to_broadcast([P, n_seq]))
    nc.gpsimd.dma_start(mr_bc[:], dln[1:2, :].to_broadcast([P, n_seq]))
    for m in range(NCH):
        nc.vector.tensor_tensor(y_sb[:, m, :], z_sb[:, m, :], rstd_bc[:],
                                ALU.mult)
        nc.vector.tensor_tensor(y_sb[:, m, :], y_sb[:, m, :], mr_bc[:],
                                ALU.subtract)
        if scale_gb is not None:
            g_sb, b_sb = scale_gb
            nc.vector.tensor_scalar(
                y_sb[:, m, :], y_sb[:, m, :],
                g_sb[:, m:m + 1], b_sb[:, m:m + 1], op0=ALU.mult, op1=ALU.add)


def _exp_slices(layer, j, n_q):
    if layer == 1:
        jlo = j * P < 512
        s_lo = W * INV_SQRT if jlo else INV_SQRT
        s_hi = INV_SQRT if jlo else W * INV_SQRT
        return [(0, 512, s_lo), (512, n_q, s_hi)]
    else:
        blk = j // 2
        return [(b * 256, min((b + 1) * 256, n_q),
                 W * INV_SQRT if b == blk else INV_SQRT) for b in range(3)]


def _emit_attn(nc, cx, work, psum_s, psum_ot, qT_sb, kT_sb, v_sb, ot_sb,
               layer, n_q, n_kv):
    sb, dram = cx.sb, cx.dram
    J = n_kv // P
    dsums = dram.tile([H, n_q], F32, tag="dsums")
    for c in range(H // 2):
        po_all = {}
        for hb, h in ((slice(0, 64), 2 * c), (slice(64, 128), 2 * c + 1)):
            po_all[h] = psum_ot.tile([65, 768], F32, tag="ot", name=f"po{h}")
        for j in range(J):
            exps = {}
            for hb, h in ((slice(0, 64), 2 * c), (slice(64, 128), 2 * c + 1)):
                e = work.tile([P, n_q], F32R, tag="exps")
                pss = []
                for (a, b) in _regions(n_q):
                    ps = psum_s.tile([P, 512], F32, tag="s")
                    nc.tensor.matmul(
                        ps[:, :b - a],
                        kT_sb[hb, c, j * P:(j + 1) * P],
                        qT_sb[hb, c, a:b],
                        start=True, stop=True)
                    pss.append((a, b, ps))
                for (a, b, ps) in pss:
                    for (lo, hi, sc) in _exp_slices(layer, j, n_q):
                        lo2, hi2 = max(lo, a), min(hi, b)
                        if lo2 < hi2:
                            nc.scalar.activation(
                                e[:, lo2:hi2], ps[:, lo2 - a:hi2 - a],
                                AF.Exp, bias=0.0, scale=sc)
                exps[h] = e
            for hb, h in ((slice(0, 64), 2 * c), (slice(64, 128), 2 * c + 1)):
                for (a, b) in _regions(n_q):
                    nc.tensor.matmul(
                        po_all[h][0:65, a:b],
                        v_sb[:, j, h, 0:65],
                        exps[h][:, a:b],
                        start=(j == 0), stop=(j == J - 1))
        for hb, h in ((slice(0, 64), 2 * c), (slice(64, 128), 2 * c + 1)):
            nc.vector.tensor_copy(ot_sb[hb, c, :], po_all[h][0:64, :n_q])
            srow = work.tile([1, n_q], F32, tag="sumrow", bufs=2,
                             name=f"srow{h}")
            nc.vector.tensor_copy(srow[:], po_all[h][64:65, :n_q])
            nc.sync.dma_start(dsums[h:h + 1, :], srow[:])
    sums_sb = sb.tile([H, n_q], F32, tag="sums_sb")
    nc.sync.dma_start(sums_sb[:], dsums[:])
    recip = sb.tile([H, n_q], F32, tag="recip")
    nc.vector.reciprocal_approx_fast(out=recip[:], in_=sums_sb[:])
    drec = dram.tile([H, n_q], F32, tag="drec")
    nc.sync.dma_start(drec[:], recip[:])
    for c in range(H // 2):
        bc = work.tile([P, n_q], F32, tag="attn_bc")
        nc.gpsimd.dma_start(bc[0:64, :],
                            drec[2 * c:2 * c + 1, :].to_broadcast([64, n_q]))
        nc.gpsimd.dma_start(bc[64:128, :],
                            drec[2 * c + 1:2 * c + 2, :].to_broadcast([64, n_q]))
        nc.vector.tensor_tensor(ot_sb[:, c, :], ot_sb[:, c, :], bc[:], ALU.mult)


def _r3(ap):
    return ap.rearrange("(ko p) s -> p ko s", p=P)


def _build_program(flags):
    use_bo1, use_bo2, use_fb1, use_fb2, use_g1, use_g2, use_g3 = flags
    nc = bacc.Bacc("TRN2", target_bir_lowering=False, debug=False)

    def din(name, shape, dt=F32R):
        return nc.dram_tensor(name, shape, dt, kind="ExternalInput").ap()

    x1T = din("x1T", [D, S1])
    x2T = din("x2T", [D, S2])
    x3T = din("x3T", [D, S3])
    wts = {n: din(n, [D, D]) for n in
           ("wq1T", "wk1T", "wv1T", "wo1T", "wq2T", "wk2T", "wv2T", "wo2T")}
    fw1T = din("fw1T", [D, DF])
    fw2T = din("fw2T", [DF, D])
    onesd = din("onesd", [P, 1])
    vones = din("vones", [P, H])
    bo1 = din("bo1", [P, NCH], F32) if use_bo1 else None
    bo2 = din("bo2", [P, NCH], F32) if use_bo2 else None
    fb1 = din("fb1", [P, DF // P], F32) if use_fb1 else None
    fb2 = din("fb2", [P, NCH], F32) if use_fb2 else None
    gbd = {}
    for i, use in ((1, use_g1), (2, use_g2), (3, use_g3)):
        gbd[i] = (din(f"g{i}", [P, NCH], F32),
                  din(f"b{i}", [P, NCH], F32)) if use else None
    yT = nc.dram_tensor("yT", [D, S1], F32, kind="ExternalOutput").ap()

    with tile.TileContext(nc, pool_alloc_mode="queue") as tc:
        cx = _Ctx()
        with tc.tile_pool(name="sb", bufs=1) as sb, \
             tc.tile_pool(name="zsq", bufs=2) as zsq_pool, \
             tc.tile_pool(name="dram", bufs=2, space="DRAM") as dram:
            cx.sb, cx.dram, cx.zsq_pool = sb, dram, zsq_pool

            ones_sb = sb.tile([P, 1], F32R, tag="ones")
            nc.sync.dma_start(ones_sb[:], onesd)
            cx.ones_sb = ones_sb
            eps_sb = sb.tile([P, 1], F32, tag="eps")
            nc.vector.memset(eps_sb[:], EPS)
            cx.eps_sb = eps_sb

            def load_pm(ap, cols, tag):
                if ap is None:
                    return None
                t = sb.tile([P, cols], F32, tag=tag)
                nc.sync.dma_start(t[:], ap)
                return t

            bo1_sb = load_pm(bo1, NCH, "bo1")
            bo2_sb = load_pm(bo2, NCH, "bo2")
            fb1_sb = load_pm(fb1, DF // P, "fb1")
            fb2_sb = load_pm(fb2, NCH, "fb2")
            gb_sb = {}
            for i in (1, 2, 3):
                gb_sb[i] = None if gbd[i] is None else (
                    load_pm(gbd[i][0], NCH, f"g{i}"),
                    load_pm(gbd[i][1], NCH, f"b{i}"))

            x1_sb = sb.tile([P, NCH, S1], F32R, tag="x1")
            nc.sync.dma_start(x1_sb[:], _r3(x1T))
            y1_sb = sb.tile([P, NCH, S1], F32R, tag="y1")
            y2_sb = sb.tile([P, NCH, S1], F32R, tag="y2")

            def attn_block(layer, xq_sb, xkv_ap, n_kv, wq, wk, wv, wo,
                           resid_sb, bias_sb, y_out, scale_gb):
                Jkv = n_kv // P
                with tc.tile_pool(name=f"kv{layer}", bufs=1) as kvp, \
                     tc.tile_pool(name=f"at{layer}", bufs=1) as atp:
                    xkv_sb = kvp.tile([P, NCH, n_kv], F32R, tag="xkv")
                    nc.sync.dma_start(xkv_sb[:], _r3(xkv_ap))
                    wq_sb = kvp.tile([P, NCH, D], F32R, tag="wq")
                    wk_sb = kvp.tile([P, NCH, D], F32R, tag="wk")
                    wv_sb = kvp.tile([P, NCH, D], F32R, tag="wv")
                    nc.sync.dma_start(wq_sb[:], _r3(wq))
                    nc.sync.dma_start(wk_sb[:], _r3(wk))
                    nc.sync.dma_start(wv_sb[:], _r3(wv))

                    q_sb = atp.tile([P, NCH, S1], F32R, tag="q")
                    k_sb = atp.tile([P, NCH, n_kv], F32R, tag="k")
                    v_sb = atp.tile([P, Jkv, H, 65], F32R, tag="v")
                    ot_sb = atp.tile([P, NCH, S1], F32R, tag="ot")

                    with tc.tile_pool(name=f"psA{layer}", bufs=3,
                                      space="PSUM") as psA:
                        _emit_proj(nc, psA, wq_sb, xq_sb, D, S1, NCH,
                                   lambda m, ps: nc.vector.tensor_copy(
                                       q_sb[:, m, :], ps))
                        _emit_proj(nc, psA, wk_sb, xkv_sb, D, n_kv, NCH,
                                   lambda m, ps: nc.vector.tensor_copy(
                                       k_sb[:, m, :], ps))
                        for j in range(Jkv):
                            nc.sync.dma_start(v_sb[:, j, :, 64:65],
                                              vones[:, :, None])
                            ps = psA.tile([P, 1024], F32, tag="proj")
                            for ko in range(NCH):
                                nc.tensor.matmul(
                                    ps[:, 0:D],
                                    xkv_sb[:, ko, j * P:(j + 1) * P],
                                    wv_sb[:, ko, :],
                                    start=(ko == 0), stop=(ko == NCH - 1))
                            nc.vector.tensor_copy(
                                v_sb[:, j, :, 0:64],
                                ps[:, 0:D].rearrange("p (h v) -> p h v", h=H))

                    with tc.tile_pool(name=f"wk{layer}", bufs=3) as work, \
                         tc.tile_pool(name=f"ps_s{layer}", bufs=4,
                                      space="PSUM") as pss, \
                         tc.tile_pool(name=f"ps_ot{layer}", bufs=2,
                                      space="PSUM") as psot:
                        _emit_attn(nc, cx, work, pss, psot, q_sb, k_sb, v_sb,
                                   ot_sb, layer, S1, n_kv)

                    with tc.tile_pool(name=f"pwo{layer}", bufs=1) as pwo, \
                         tc.tile_pool(name=f"psB{layer}", bufs=2,
                                      space="PSUM") as psB:
                        wo_sb = pwo.tile([P, NCH, D], F32R, tag="wo")
                        nc.sync.dma_start(wo_sb[:], _r3(wo))
                        wo_ps = []
                        _emit_proj(nc, psB, wo_sb, ot_sb, D, S1, NCH,
                                   lambda m, ps: wo_ps.append(ps))
                        _emit_ln(nc, cx, psB, lambda m: wo_ps[m], resid_sb,
                                 bias_sb, y_out, S1, scale_gb)

            attn_block(1, x1_sb, x2T, S2, wts["wq1T"], wts["wk1T"],
                       wts["wv1T"], wts["wo1T"], x1_sb, bo1_sb, y1_sb,
                       gb_sb[1])
            attn_block(2, y1_sb, x3T, S3, wts["wq2T"], wts["wk2T"],
                       wts["wv2T"], wts["wo2T"], y1_sb, bo2_sb, y2_sb,
                       gb_sb[2])

            yT_sb = sb.tile([P, NCH, S1], F32, tag="y1")
            with tc.tile_pool(name="ffn1", bufs=1) as f1p:
                fw1_sb = f1p.tile([P, NCH, DF], F32R, tag="fw1")
                nc.sync.dma_start(fw1_sb[:], _r3(fw1T))
                h_sb = f1p.tile([P, DF // P, S1], F32R, tag="hT")
                with tc.tile_pool(name="psE", bufs=3, space="PSUM") as psE:
                    def gelu_consume(m, ps):
                        for (a, b) in _regions(S1):
                            nc.scalar.activation(
                                h_sb[:, m, a:b], ps[:, a:b], AF.Gelu,
                                bias=(fb1_sb[:, m:m + 1]
                                      if fb1_sb is not None else 0.0),
                                scale=1.0)
                    _emit_proj(nc, psE, fw1_sb, y2_sb, DF, S1, NCH,
                               gelu_consume)

                with tc.tile_pool(name="ffn2", bufs=1) as f2p, \
                     tc.tile_pool(name="psF", bufs=2, space="PSUM") as psF:
                    fw2_sb = f2p.tile([P, DF // P, D], F32R, tag="fw2")
                    nc.sync.dma_start(fw2_sb[:], _r3(fw2T))
                    f2_ps = []
                    _emit_proj(nc, psF, fw2_sb, h_sb, D, S1, DF // P,
                               lambda m, ps: f2_ps.append(ps))
                    _emit_ln(nc, cx, psF, lambda m: f2_ps[m], y2_sb, fb2_sb,
                             yT_sb, S1, gb_sb[3])
            nc.sync.dma_start(_r3(yT), yT_sb[:])

    nc.finalize()
    return nc


def _to_pm(vec, cols):
    return np.ascontiguousarray(vec.reshape(cols, P).T).astype(np.float32)


def kernel(**inputs):
    cords = np.asarray(inputs["cords_features"], np.float32)
    spatial = np.asarray(inputs["spatial_features"], np.float32)
    speed = np.asarray(inputs["speed_features"], np.float32)
    B = cords.shape[0]
    assert B == 8

    def g(name):
        return np.asarray(inputs[name], np.float32)

    flags = (
        not np.allclose(g("bo1"), 0), not np.allclose(g("bo2"), 0),
        not np.allclose(g("ffn_b1"), 0), not np.allclose(g("ffn_b2"), 0),
        not (np.allclose(g("ln1_g"), 1) and np.allclose(g("ln1_b"), 0)),
        not (np.allclose(g("ln2_g"), 1) and np.allclose(g("ln2_b"), 0)),
        not (np.allclose(g("ln3_g"), 1) and np.allclose(g("ln3_b"), 0)),
    )
    if flags not in _PROGRAM_CACHE:
        _PROGRAM_CACHE[flags] = _build_program(flags)
    nc = _PROGRAM_CACHE[flags]

    shared = {
        "wq1T": np.ascontiguousarray(g("wq1").T),
        "wk1T": np.ascontiguousarray(g("wk1").T),
        "wv1T": np.ascontiguousarray(g("wv1").T),
        "wo1T": np.ascontiguousarray(g("wo1").T),
        "wq2T": np.ascontiguousarray(g("wq2").T),
        "wk2T": np.ascontiguousarray(g("wk2").T),
        "wv2T": np.ascontiguousarray(g("wv2").T),
        "wo2T": np.ascontiguousarray(g("wo2").T),
        "fw1T": np.ascontiguousarray(g("ffn_w1").T),
        "fw2T": np.ascontiguousarray(g("ffn_w2").T),
        "onesd": np.ones((P, 1), np.float32),
        "vones": np.ones((P, H), np.float32),
    }
    use_bo1, use_bo2, use_fb1, use_fb2, use_g1, use_g2, use_g3 = flags
    if use_bo1:
        shared["bo1"] = _to_pm(g("bo1"), NCH)
    if use_bo2:
        shared["bo2"] = _to_pm(g("bo2"), NCH)
    if use_fb1:
        shared["fb1"] = _to_pm(g("ffn_b1"), DF // P)
    if use_fb2:
        shared["fb2"] = _to_pm(g("ffn_b2"), NCH)
    for i, use in ((1, use_g1), (2, use_g2), (3, use_g3)):
        if use:
            shared[f"g{i}"] = _to_pm(g(f"ln{i}_g"), NCH)
            shared[f"b{i}"] = _to_pm(g(f"ln{i}_b"), NCH)

    in_maps = []
    for b in range(B):
        m = dict(shared)
        m["x1T"] = np.ascontiguousarray(cords[b].T)
        m["x2T"] = np.ascontiguousarray(spatial[b].T)
        m["x3T"] = np.ascontiguousarray(speed[b].T)
        in_maps.append(m)

    global _LAST_IN_MAPS
    _LAST_IN_MAPS = in_maps
    res = run_bass_kernel_spmd(nc, in_maps, core_ids=list(range(B)))
    out = np.stack([res.results[b]["yT"].T for b in range(B)], axis=0)
    return np.ascontiguousarray(out.astype(np.float32))
